# revision 24
# baseline (speedup 1.0000x reference)
"""Trainium2 Bass kernel for nn_Attention_35588099015470.

Full transformer attention block: LoRA linears (folded host-side) + RoPE +
causal SDPA + output projection, B=2 T=2048 C=2048 H=16 D=128, fp32 in/out.

Sharding: tensor-parallel over heads — 8 cores x 2 heads. All matmul operands
are bf16 (fp32 PSUM accumulation): same PE rate as fp32r on this hardware but
half the DMA/SBUF footprint, which lets q/k/v live entirely in SBUF between
the projection phase and attention (no DRAM spill round-trip).

Phase A computes q/k/v for the core's 2 heads in transposed [feature, token]
layout (RoPE fused on the DVE), writing straight into persistent SBUF tiles;
v is PE-transposed to natural [token, d] layout for the PV stationary.
Phase B runs causal attention per (batch, head) in [key, query] score layout:
score matmuls land two key chunks in one 2-bank PSUM tile so a single Act
exp covers 1024 columns (the Act engine otherwise paces the pipeline);
diagonal-band chunks are causally masked with a 0/1 multiply on the DVE.
Ones-matmul column sums + PV accumulate per query tile; normalization is two
DVE ops (reciprocal of the full colsum PSUM tile - every partition already
holds the sum - then multiply), feeding an AllToAll per (batch, head) that
reshards head-parallel -> token-parallel. Pair order (0,0),(1,0),(0,1),(1,1)
lets the hl=0 half of the output projection run between attention pairs
(partials in bf16 SBUF) and the hl=1/batch-0 quarter fill the last
AllToAll's latency, so only the hl=1/batch-1 quarter remains in the tail.

Biases are guaranteed zero by the problem's setup_inputs and the mask is the
causal tril; if either assumption is violated at runtime we fall back to a
host reference implementation so the kernel stays correct on any input.
"""
import sys

sys.path.insert(0, "/opt/trn_rl_repo")

import numpy as np
import ml_dtypes
from contextlib import ExitStack

import concourse.tile as tile
from concourse import bacc, mybir
from concourse.bass_utils import run_bass_kernel_spmd

dt = mybir.dt
MMDT = dt.bfloat16

B, T, C, H, R = 2, 2048, 2048, 16, 8
D = C // H            # 128
NCORES = 8
HPC = H // NCORES     # heads per core = 2
P = 128
TT = (B * T) // 512   # 8 token tiles of 512
KC = C // P           # 16 contraction chunks
QT = T // 512         # 4 query tiles per (b, h)
SCALE = 1.0 / float(np.sqrt(D))
BT = B * T

_PROGRAM = None


def _build_program():
    nc = bacc.Bacc("TRN2", target_bir_lowering=False, debug=False,
                   num_devices=NCORES)

    xT_d = nc.dram_tensor("xT", [C, BT], MMDT, kind="ExternalInput")
    wqT_d = nc.dram_tensor("wqT", [C, HPC * D], MMDT, kind="ExternalInput")
    wkT_d = nc.dram_tensor("wkT", [C, HPC * D], MMDT, kind="ExternalInput")
    wvT_d = nc.dram_tensor("wvT", [C, HPC * D], MMDT, kind="ExternalInput")
    # [co, hl, p, r, m]: phase-C weights, hl-major so each half streams whole
    pwB_d = nc.dram_tensor("pwB", [KC, HPC, P, NCORES, P], MMDT,
                           kind="ExternalInput")
    cosA_d = nc.dram_tensor("cosA", [P, BT], MMDT, kind="ExternalInput")
    sinA_d = nc.dram_tensor("sinA", [P, BT], MMDT, kind="ExternalInput")
    mask01_d = nc.dram_tensor("mask01", [4, P, 512], MMDT, kind="ExternalInput")
    ident_d = nc.dram_tensor("ident", [P, P], MMDT, kind="ExternalInput")

    outT_d = nc.dram_tensor("outT", [C, 512], MMDT, kind="ExternalOutput")

    with tile.TileContext(nc) as tc, ExitStack() as ctx:
        dram = ctx.enter_context(tc.tile_pool(name="dram", bufs=1, space="DRAM"))
        # A2A staging: one collective per (batch, head-local)
        chs = [[dram.tile([NCORES, D, 256], MMDT, name=f"ch_{b}_{hl}")
                for hl in range(HPC)] for b in range(B)]
        yos = [[dram.tile([NCORES * D, 256], MMDT, name=f"yo_{b}_{hl}")
                for hl in range(HPC)] for b in range(B)]

        # persistent SBUF (live across phases)
        cst = ctx.enter_context(tc.tile_pool(name="cst", bufs=1))
        qT_all = cst.tile([P, HPC, BT], MMDT, name="qT_all")
        kT_all = cst.tile([P, HPC, BT], MMDT, name="kT_all")
        v_all = cst.tile([P, TT * 4, HPC * D], MMDT, name="v_all")
        cpart = cst.tile([P, KC, 512], MMDT, name="cpart")
        yAB = cst.tile([P, HPC, NCORES, 512], MMDT, name="yAB")

        ones_f = cst.tile([P, P], dt.float32, name="ones_f")
        nc.any.memset(ones_f[:], 1.0)
        ones_r = cst.tile([P, P], MMDT, name="ones_r")
        nc.vector.tensor_copy(ones_r[:], ones_f[:])
        ident = cst.tile([P, P], MMDT, name="ident")
        mask01 = cst.tile([P, 4, 512], MMDT, name="mask01")

        # ---------------- Phase A: q/k/v projections + RoPE -----------------
        with tc.tile_pool(name="pa_w", bufs=1) as wp, \
             tc.tile_pool(name="pa_x", bufs=2) as xp, \
             tc.tile_pool(name="pa_cs", bufs=3) as csp, \
             tc.tile_pool(name="pa_tmp", bufs=3) as tp, \
             tc.tile_pool(name="pa_vt", bufs=3) as vtp, \
             tc.tile_pool(name="pa_ps", bufs=1, space="PSUM") as pp:

            xT_view = xT_d.ap().rearrange("(a p) t -> p a t", p=P)
            wq_sb = wp.tile([P, KC, HPC * D], MMDT, name="wq_sb")
            wk_sb = wp.tile([P, KC, HPC * D], MMDT, name="wk_sb")
            wv_sb = wp.tile([P, KC, HPC * D], MMDT, name="wv_sb")
            # first x group + first wq group first, so the PE starts ASAP
            xt0 = xp.tile([P, KC, 512], MMDT, name="xt_0", tag="xt")
            wq_view = wqT_d.ap().rearrange("(a p) m -> p a m", p=P)
            nc.sync.dma_start(xt0[:, 0:1, :], xT_view[:, 0:1, 0:512])
            nc.sync.dma_start(wq_sb[:, 0:1, :], wq_view[:, 0:1, :])
            nc.sync.dma_start(xt0[:, 1:4, :], xT_view[:, 1:4, 0:512])
            nc.sync.dma_start(wq_sb[:, 1:4, :], wq_view[:, 1:4, :])
            for g in range(1, 4):
                nc.sync.dma_start(xt0[:, g * 4:(g + 1) * 4, :],
                                  xT_view[:, g * 4:(g + 1) * 4, 0:512])
                nc.sync.dma_start(wq_sb[:, g * 4:(g + 1) * 4, :],
                                  wq_view[:, g * 4:(g + 1) * 4, :])
            for w_sb, wd in ((wk_sb, wkT_d), (wv_sb, wvT_d)):
                wv_view = wd.ap().rearrange("(a p) m -> p a m", p=P)
                for g in range(4):
                    nc.sync.dma_start(w_sb[:, g * 4:(g + 1) * 4, :],
                                      wv_view[:, g * 4:(g + 1) * 4, :])
            nc.sync.dma_start(ident[:], ident_d.ap())
            for o in range(4):
                nc.sync.dma_start(mask01[:, o, :], mask01_d.ap()[o])

            for tt in range(TT):
                tsl = slice(tt * 512, (tt + 1) * 512)
                if tt == 0:
                    xt = xt0
                else:
                    xt = xp.tile([P, KC, 512], MMDT, name=f"xt_{tt}", tag="xt")
                    for g in range(4):
                        nc.sync.dma_start(xt[:, g * 4:(g + 1) * 4, :],
                                          xT_view[:, g * 4:(g + 1) * 4, tsl])
                cs_c = csp.tile([P, 512], MMDT, tag="csc", name=f"csc_{tt}")
                nc.sync.dma_start(cs_c[:], cosA_d.ap()[:, tsl])
                cs_s = csp.tile([P, 512], MMDT, tag="css", name=f"css_{tt}")
                nc.sync.dma_start(cs_s[:], sinA_d.ap()[:, tsl])

                for w_sb, dst in ((wq_sb, qT_all), (wk_sb, kT_all)):
                    for mt in range(HPC):
                        ps = pp.tile([P, 512], dt.float32, tag="qk", bufs=6,
                                     name=f"psA_{tt}_{mt}")
                        for kc in range(KC):
                            nc.tensor.matmul(
                                ps[:], w_sb[:, kc, mt * P:(mt + 1) * P],
                                xt[:, kc, :],
                                start=(kc == 0), stop=(kc == KC - 1))
                        # rope: y = raw*cosA + halfswap(raw)*sinA
                        t1 = tp.tile([P, 512], dt.float32, tag="t1",
                                     name=f"t1_{tt}_{mt}")
                        nc.vector.tensor_mul(t1[:], ps[:], cs_c[:])
                        t2 = tp.tile([P, 512], dt.float32, tag="t2",
                                     name=f"t2_{tt}_{mt}")
                        nc.vector.tensor_mul(t2[0:64, :], ps[64:128, :],
                                             cs_s[0:64, :])
                        nc.vector.tensor_mul(t2[64:128, :], ps[0:64, :],
                                             cs_s[64:128, :])
                        nc.vector.tensor_add(dst[:, mt, tsl], t1[:], t2[:])

                # v computed transposed (N=512 matmuls), then PE-transposed
                # back to natural [token, d] layout for the PV stationary
                for mt in range(HPC):
                    ps = pp.tile([P, 512], dt.float32, tag="qk", bufs=6,
                                 name=f"psVT_{tt}_{mt}")
                    for kc in range(KC):
                        nc.tensor.matmul(
                            ps[:], wv_sb[:, kc, mt * P:(mt + 1) * P],
                            xt[:, kc, :],
                            start=(kc == 0), stop=(kc == KC - 1))
                    vT_sb = vtp.tile([P, 512], MMDT, tag="vts",
                                     name=f"vts_{tt}_{mt}")
                    nc.scalar.copy(vT_sb[:], ps[:])
                    for js in range(4):
                        pst = pp.tile([P, P], MMDT, tag="tp", bufs=2,
                                      name=f"pst_{tt}_{mt}_{js}")
                        nc.tensor.transpose(pst[:], vT_sb[:, js * P:(js + 1) * P],
                                            ident[:])
                        dst_v = v_all[:, tt * 4 + js, mt * P:(mt + 1) * P]
                        if js % 2 == 0:
                            nc.scalar.copy(dst_v, pst[:])
                        else:
                            nc.vector.tensor_copy(dst_v, pst[:])

        # ---------------- Phase B + C: attention, A2A, out-projection ------
        with tc.tile_pool(name="pb_p", bufs=6) as ppool, \
             tc.tile_pool(name="pb_nm", bufs=2) as nmp, \
             tc.tile_pool(name="pc_w", bufs=4) as pwp, \
             tc.tile_pool(name="pc_o", bufs=3) as ocp, \
             tc.tile_pool(name="pb_ps", bufs=1, space="PSUM") as pb:

            def attention_pair(b, hl):
                kT_h = kT_all[:, hl, b * T:(b + 1) * T]
                qT_h = qT_all[:, hl, b * T:(b + 1) * T]
                v_h = v_all[:, b * 16:(b + 1) * 16, hl * D:(hl + 1) * D]

                for qt in range(QT):
                    qTt = qT_h[:, qt * 512:(qt + 1) * 512]
                    n = 4 * (qt + 1)
                    smps = pb.tile([P, 512], dt.float32, tag="sm", bufs=2,
                                   name=f"sm_{b}_{hl}_{qt}")
                    pvps = pb.tile([P, 512], dt.float32, tag="pv", bufs=2,
                                   name=f"pv_{b}_{hl}_{qt}")

                    sc_tiles = {}

                    def emit_sc2(jp, _qt=qt, _q=qTt, _n=n, _sc=sc_tiles,
                                 _b=b, _hl=hl):
                        # two key chunks per 2-bank PSUM tile -> one exp
                        ps = pb.tile([P, 2, 512], dt.float32, tag="sc", bufs=2,
                                     name=f"sc_{_b}_{_hl}_{_qt}_{jp}")
                        for u in (0, 1):
                            jc = 2 * jp + u
                            nc.tensor.matmul(ps[:, u, :],
                                             kT_h[:, jc * P:(jc + 1) * P],
                                             _q[:], start=True, stop=True)
                        _sc[jp] = ps

                    pT_tiles = {}

                    def emit_exp(jp, _qt=qt, _n=n, _b=b, _hl=hl):
                        scps = sc_tiles.pop(jp)
                        pT2 = ppool.tile([P, 2, 512], MMDT, tag="pT",
                                         name=f"pT_{_b}_{_hl}_{_qt}_{jp}")
                        nc.scalar.activation(pT2[:], scps[:],
                                             mybir.ActivationFunctionType.Exp,
                                             scale=SCALE)
                        # diagonal-band chunks: zero the causally-masked
                        # entries with a 0/1 mask on the DVE (cheaper than
                        # masking matmuls on the PE)
                        for u in (0, 1):
                            jc = 2 * jp + u
                            if jc >= _n - 4:
                                o = jc - (_n - 4)
                                nc.vector.tensor_mul(pT2[:, u, :], pT2[:, u, :],
                                                     mask01[:, o, :])
                        pT_tiles[jp] = pT2

                    # two chunk-pairs of lookahead so exp latency stays off
                    # the PE critical path
                    emit_sc2(0)
                    emit_exp(0)
                    if n // 2 > 1:
                        emit_sc2(1)
                        emit_exp(1)
                    for jp in range(n // 2):
                        pT2 = pT_tiles.pop(jp)
                        for u in (0, 1):
                            jc = 2 * jp + u
                            nc.tensor.matmul(smps[:], ones_r[:], pT2[:, u, :],
                                             start=(jc == 0), stop=(jc == n - 1))
                            nc.tensor.matmul(pvps[:], v_h[:, jc, :],
                                             pT2[:, u, :],
                                             start=(jc == 0), stop=(jc == n - 1))
                        if jp + 2 < n // 2:
                            emit_sc2(jp + 2)
                            emit_exp(jp + 2)

                    # normalize: all 128 partitions of smps hold the colsum,
                    # so reciprocal of the full tile is a pre-broadcast scale
                    rec = nmp.tile([P, 512], dt.float32, tag="rec",
                                   name=f"rec_{b}_{hl}_{qt}")
                    nc.vector.reciprocal(rec[:], smps[:])
                    yt = nmp.tile([P, 512], MMDT, tag="yt",
                                  name=f"yt_{b}_{hl}_{qt}")
                    nc.vector.tensor_mul(yt[:], pvps[:], rec[:])
                    nc.sync.dma_start(chs[b][hl][2 * qt][:, :], yt[:, 0:256])
                    nc.sync.dma_start(chs[b][hl][2 * qt + 1][:, :],
                                      yt[:, 256:512])

                nc.gpsimd.collective_compute(
                    "AllToAll", mybir.AluOpType.bypass,
                    replica_groups=[list(range(NCORES))],
                    ins=[chs[b][hl].opt()], outs=[yos[b][hl].opt()],
                )

            def emit_gathers(hlh, b, split=False):
                # A2A-gated: emit only where nothing latency-critical queues
                # behind them on the same DMA queue. split=True gathers rank
                # by rank so the consumer's r=0 matmul starts ~2.5us sooner.
                yv = yos[b][hlh][:].rearrange("(r p) t -> p r t", p=P)
                if split:
                    for r in range(NCORES):
                        nc.sync.dma_start(
                            yAB[:, hlh, r, b * 256:(b + 1) * 256], yv[:, r, :])
                else:
                    nc.sync.dma_start(yAB[:, hlh, :, b * 256:(b + 1) * 256],
                                      yv[:])

            pw_sb = {}

            def load_pw(hlh):
                # ungated streaming loads; fully resident (no ring waits)
                for co in range(KC):
                    pw = pwp.tile([P, NCORES, P], MMDT, tag=f"pw{hlh}",
                                  bufs=KC, name=f"pw_{hlh}_{co}")
                    nc.sync.dma_start(pw[:], pwB_d.ap()[co, hlh])
                    pw_sb[(hlh, co)] = pw

            def cpass0():
                """hl=0 half of the output projection -> bf16 partials."""
                for cp in range(KC // 2):
                    pso = pb.tile([P, 2, 512], dt.float32, tag="sc", bufs=2,
                                  name=f"pso_0_{cp}")
                    for u in (0, 1):
                        co = 2 * cp + u
                        pw = pw_sb[(0, co)]
                        for r in range(NCORES):
                            nc.tensor.matmul(pso[:, u, :], pw[:, r, :],
                                             yAB[:, 0, r, :],
                                             start=(r == 0),
                                             stop=(r == NCORES - 1))
                    for u in (0, 1):
                        co = 2 * cp + u
                        if u == 0:
                            nc.scalar.copy(cpart[:, co, :], pso[:, u, :])
                        else:
                            nc.vector.tensor_copy(cpart[:, co, :], pso[:, u, :])

            def cpass1_half(bh):
                """hl=1 half of the output projection for batch bh columns,
                combined with the hl=0 partials and written out. The bh=0
                half only needs a2a(0,1), so it fills the a2a(1,1) wait."""
                csl = slice(bh * 256, (bh + 1) * 256)
                for cp in range(KC // 2):
                    pso = pb.tile([P, 2, 512], dt.float32, tag="sc", bufs=2,
                                  name=f"pso1_{bh}_{cp}")
                    for u in (0, 1):
                        co = 2 * cp + u
                        pw = pw_sb[(1, co)]
                        for r in range(NCORES):
                            nc.tensor.matmul(pso[:, u, csl], pw[:, r, :],
                                             yAB[:, 1, r, csl],
                                             start=(r == 0),
                                             stop=(r == NCORES - 1))
                    for u in (0, 1):
                        co = 2 * cp + u
                        oo = ocp.tile([P, 256], MMDT, tag="ooh", bufs=4,
                                      name=f"oo_{bh}_{co}")
                        nc.vector.tensor_add(oo[:], pso[:, u, csl],
                                             cpart[:, co, csl])
                        nc.sync.dma_start(
                            outT_d.ap()[co * P:(co + 1) * P, csl], oo[:])

            attention_pair(0, 0)
            attention_pair(1, 0)
            attention_pair(0, 1)
            load_pw(0)
            load_pw(1)
            emit_gathers(0, 0)
            emit_gathers(0, 1)
            cpass0()                   # hl=0 half, hidden behind pair (1,1)
            emit_gathers(1, 0)
            attention_pair(1, 1)
            emit_gathers(1, 1, split=True)
            cpass1_half(0)             # fills the a2a(1,1) wait
            cpass1_half(1)             # collective tail: only this remains

    nc.compile()
    return nc


def _host_reference(x, weights, cos, sin, mask, use_lora):
    """Numpy fallback for inputs outside the optimized assumptions."""
    (q_w, q_b, q_A, q_B, k_w, k_b, k_A, k_B,
     v_w, v_b, v_A, v_B, p_w, p_b, p_A, p_B) = weights

    def lin(xx, w, b, A, Bm):
        out = xx @ w.T + b
        if use_lora:
            out = out + (xx @ A) @ Bm
        return out

    def rope(t):
        x1, x2 = t[..., ::2], t[..., 1::2]
        y = np.stack((x1 * cos - x2 * sin, x1 * sin + x2 * cos), axis=-1)
        return y.reshape(t.shape)

    Bs, Tl, Cd = x.shape
    q = lin(x, q_w, q_b, q_A, q_B).reshape(Bs, Tl, H, D).transpose(0, 2, 1, 3)
    k = lin(x, k_w, k_b, k_A, k_B).reshape(Bs, Tl, H, D).transpose(0, 2, 1, 3)
    v = lin(x, v_w, v_b, v_A, v_B).reshape(Bs, Tl, H, D).transpose(0, 2, 1, 3)
    q, k = rope(q), rope(k)
    s = np.einsum('bhqd,bhkd->bhqk', q, k) / np.sqrt(D)
    s = np.where(mask, s, -np.inf)
    s = s - s.max(axis=-1, keepdims=True)
    p = np.exp(s)
    p /= p.sum(axis=-1, keepdims=True)
    o = np.einsum('bhqk,bhkd->bhqd', p, v).transpose(0, 2, 1, 3).reshape(Bs, Tl, Cd)
    return lin(o, p_w, p_b, p_A, p_B).astype(np.float32)


def kernel(**inputs):
    x = np.asarray(inputs["x"], np.float32)
    cos = np.asarray(inputs["cos"], np.float32)
    sin = np.asarray(inputs["sin"], np.float32)
    mask = np.asarray(inputs["mask"])
    use_lora = int(np.asarray(inputs["use_lora"]))
    ws = {}
    for nm in ("q", "k", "v", "p"):
        for suf in ("w", "b", "A", "B"):
            ws[f"{nm}_{suf}"] = np.asarray(inputs[f"{nm}_{suf}"], np.float32)

    causal = bool((mask == np.tril(np.ones((T, T), bool))).all())
    zero_bias = all(not ws[f"{nm}_b"].any() for nm in ("q", "k", "v", "p"))
    if not (causal and zero_bias and x.shape == (B, T, C)):
        weights = tuple(ws[f"{nm}_{suf}"] for nm in ("q", "k", "v", "p")
                        for suf in ("w", "b", "A", "B"))
        return _host_reference(x, weights, cos, sin, mask, use_lora)

    # effective (LoRA-folded) transposed weights: out = x @ W_eff.T,
    # W_eff.T = w.T + A @ B
    effT = {}
    for nm in ("q", "k", "v", "p"):
        wt = ws[f"{nm}_w"].T.copy()
        if use_lora:
            wt += ws[f"{nm}_A"] @ ws[f"{nm}_B"]
        effT[nm] = np.ascontiguousarray(wt, np.float32)

    xT = np.ascontiguousarray(x.reshape(BT, C).T)

    # sigma: within each head reorder out-features to [evens, odds] so the
    # rope pair-rotation becomes a partition half-swap
    perm = np.concatenate([np.arange(0, D, 2), np.arange(1, D, 2)])
    cosT = cos.T.astype(np.float32)          # [64, T]
    sinT = sin.T.astype(np.float32)
    cosA = np.tile(np.vstack([cosT, cosT]), (1, B))          # [128, B*T]
    sinA = np.tile(np.vstack([-sinT, sinT]), (1, B))

    # causal 0/1 mask for the 4 diagonal-band key chunks of each query tile:
    # mask01[o, k, q] = 1 iff key k + 128*o <= query q (multiplies exp'd
    # scores on the DVE)
    kk = np.arange(P)[:, None]
    qq = np.arange(512)[None, :]
    mask01 = np.stack([(kk + 128 * o <= qq).astype(np.float32)
                       for o in range(4)])

    # output projection weight, blocked [co, hl, p, r, m]: y-feature block for
    # head (r, hl) contracts against rows (2r+hl)*128+p of effT["p"]
    pwB = np.ascontiguousarray(
        effT["p"].reshape(NCORES, HPC, P, KC, P)     # [r, hl, p, co, m]
        .transpose(3, 1, 2, 0, 4))                   # [co, hl, p, r, m]

    ident = np.eye(P, dtype=np.float32)

    global _PROGRAM
    if _PROGRAM is None:
        _PROGRAM = _build_program()
    nc = _PROGRAM

    mmnp = mybir.dt.np(MMDT)

    in_maps = []
    xT_mm = xT.astype(mmnp)
    pwB_mm = pwB.astype(mmnp)
    for c in range(NCORES):
        cols = slice(c * HPC * D, (c + 1) * HPC * D)
        wqT = effT["q"][:, cols].copy()
        wkT = effT["k"][:, cols].copy()
        for hl in range(HPC):
            sl = slice(hl * D, (hl + 1) * D)
            wqT[:, sl] = wqT[:, sl][:, perm]
            wkT[:, sl] = wkT[:, sl][:, perm]
        in_maps.append({
            "xT": xT_mm,
            "wqT": np.ascontiguousarray(wqT).astype(mmnp),
            "wkT": np.ascontiguousarray(wkT).astype(mmnp),
            "wvT": np.ascontiguousarray(effT["v"][:, cols]).astype(mmnp),
            "pwB": pwB_mm,
            "cosA": cosA.astype(mmnp),
            "sinA": sinA.astype(mmnp),
            "mask01": mask01.astype(mmnp),
            "ident": ident.astype(mmnp),
        })

    res = run_bass_kernel_spmd(nc, in_maps, list(range(NCORES)))

    out = np.empty((BT, C), np.float32)
    for c in range(NCORES):
        oT = np.asarray(res.results[c]["outT"], np.float32)   # [2048, 512]
        out[c * 256:(c + 1) * 256, :] = oT[:, 0:256].T             # b = 0
        out[T + c * 256:T + (c + 1) * 256, :] = oT[:, 256:512].T   # b = 1
    return out.reshape(B, T, C)


# revision 25
# speedup vs baseline: 1.2017x; 1.2017x over previous
"""Trainium2 Bass kernel for nn_Attention_35588099015470.

Full transformer attention block: LoRA linears (folded host-side) + RoPE +
causal SDPA + output projection, B=2 T=2048 C=2048 H=16 D=128, fp32 in/out.

Sharding: tensor-parallel over heads — 8 cores x 2 heads. All matmul operands
are bf16 (fp32 PSUM accumulation): same PE rate as fp32r on this hardware but
half the DMA/SBUF footprint, which lets q/k/v live entirely in SBUF between
the projection phase and attention (no DRAM spill round-trip).

Phase A computes q/k/v for the core's 2 heads in transposed [feature, token]
layout (RoPE fused on the DVE), writing straight into persistent SBUF tiles;
v is PE-transposed to natural [token, d] layout for the PV stationary.
Phase B runs causal attention per (batch, head) in [key, query] score layout:
score matmuls land two key chunks in one 2-bank PSUM tile so a single Act
exp covers 1024 columns (the Act engine otherwise paces the pipeline);
diagonal-band chunks are causally masked with a 0/1 multiply on the DVE.
Ones-matmul column sums + PV accumulate per query tile; normalization is two
DVE ops (reciprocal of the full colsum PSUM tile - every partition already
holds the sum - then multiply), feeding an AllToAll per (batch, head) that
reshards head-parallel -> token-parallel. Pair order (0,0),(1,0),(0,1),(1,1)
lets the hl=0 half of the output projection run between attention pairs
(partials in bf16 SBUF) and the hl=1/batch-0 quarter fill the last
AllToAll's latency, so only the hl=1/batch-1 quarter remains in the tail.

Biases are guaranteed zero by the problem's setup_inputs and the mask is the
causal tril; if either assumption is violated at runtime we fall back to a
host reference implementation so the kernel stays correct on any input.
"""
import sys

sys.path.insert(0, "/opt/trn_rl_repo")

import numpy as np
import ml_dtypes
from contextlib import ExitStack

import concourse.tile as tile
from concourse import bacc, mybir
from concourse.bass_utils import run_bass_kernel_spmd

dt = mybir.dt
MMDT = dt.bfloat16

B, T, C, H, R = 2, 2048, 2048, 16, 8
D = C // H            # 128
NCORES = 8
HPC = H // NCORES     # heads per core = 2
P = 128
TT = (B * T) // 512   # 8 token tiles of 512
KC = C // P           # 16 contraction chunks
QT = T // 512         # 4 query tiles per (b, h)
SCALE = 1.0 / float(np.sqrt(D))
BT = B * T

_PROGRAM = None


def _build_program():
    nc = bacc.Bacc("TRN2", target_bir_lowering=False, debug=False,
                   num_devices=NCORES)

    xT_d = nc.dram_tensor("xT", [C, BT], MMDT, kind="ExternalInput")
    wqT_d = nc.dram_tensor("wqT", [C, HPC * D], MMDT, kind="ExternalInput")
    wkT_d = nc.dram_tensor("wkT", [C, HPC * D], MMDT, kind="ExternalInput")
    wvT_d = nc.dram_tensor("wvT", [C, HPC * D], MMDT, kind="ExternalInput")
    # [co, hl, p, r, m]: phase-C weights, hl-major so each half streams whole
    pwB_d = nc.dram_tensor("pwB", [KC, HPC, P, NCORES, P], MMDT,
                           kind="ExternalInput")
    cosA_d = nc.dram_tensor("cosA", [P, BT], MMDT, kind="ExternalInput")
    sinA_d = nc.dram_tensor("sinA", [P, BT], MMDT, kind="ExternalInput")
    mask01_d = nc.dram_tensor("mask01", [4, P, 512], MMDT, kind="ExternalInput")
    ident_d = nc.dram_tensor("ident", [P, P], MMDT, kind="ExternalInput")

    outT_d = nc.dram_tensor("outT", [C, 512], MMDT, kind="ExternalOutput")

    with tile.TileContext(nc) as tc, ExitStack() as ctx:
        dram = ctx.enter_context(tc.tile_pool(name="dram", bufs=1, space="DRAM"))
        # A2A staging: one collective per (batch, head-local)
        chs = [[dram.tile([NCORES, D, 256], MMDT, name=f"ch_{b}_{hl}")
                for hl in range(HPC)] for b in range(B)]
        yos = [[dram.tile([NCORES * D, 256], MMDT, name=f"yo_{b}_{hl}")
                for hl in range(HPC)] for b in range(B)]

        # persistent SBUF (live across phases)
        cst = ctx.enter_context(tc.tile_pool(name="cst", bufs=1))
        qT_all = cst.tile([P, HPC, BT], MMDT, name="qT_all")
        kT_all = cst.tile([P, HPC, BT], MMDT, name="kT_all")
        v_all = cst.tile([P, TT * 4, HPC * D], MMDT, name="v_all")
        cpart = cst.tile([P, KC, 512], MMDT, name="cpart")
        yAB = cst.tile([P, HPC, NCORES, 512], MMDT, name="yAB")

        ones_f = cst.tile([P, P], dt.float32, name="ones_f")
        nc.any.memset(ones_f[:], 1.0)
        ones_r = cst.tile([P, P], MMDT, name="ones_r")
        nc.vector.tensor_copy(ones_r[:], ones_f[:])
        ident = cst.tile([P, P], MMDT, name="ident")
        mask01 = cst.tile([P, 4, 512], MMDT, name="mask01")

        # ---------------- Phase A: q/k/v projections + RoPE -----------------
        with tc.tile_pool(name="pa_w", bufs=1) as wp, \
             tc.tile_pool(name="pa_x", bufs=2) as xp, \
             tc.tile_pool(name="pa_cs", bufs=3) as csp, \
             tc.tile_pool(name="pa_tmp", bufs=3) as tp, \
             tc.tile_pool(name="pa_vt", bufs=3) as vtp, \
             tc.tile_pool(name="pa_ps", bufs=1, space="PSUM") as pp:

            xT_view = xT_d.ap().rearrange("(a p) t -> p a t", p=P)
            wq_sb = wp.tile([P, KC, HPC * D], MMDT, name="wq_sb")
            wk_sb = wp.tile([P, KC, HPC * D], MMDT, name="wk_sb")
            wv_sb = wp.tile([P, KC, HPC * D], MMDT, name="wv_sb")
            # first x group + first wq group first, so the PE starts ASAP
            xt0 = xp.tile([P, KC, 512], MMDT, name="xt_0", tag="xt")
            wq_view = wqT_d.ap().rearrange("(a p) m -> p a m", p=P)
            nc.sync.dma_start(xt0[:, 0:1, :], xT_view[:, 0:1, 0:512])
            nc.sync.dma_start(wq_sb[:, 0:1, :], wq_view[:, 0:1, :])
            nc.sync.dma_start(xt0[:, 1:4, :], xT_view[:, 1:4, 0:512])
            nc.sync.dma_start(wq_sb[:, 1:4, :], wq_view[:, 1:4, :])
            for g in range(1, 4):
                nc.sync.dma_start(xt0[:, g * 4:(g + 1) * 4, :],
                                  xT_view[:, g * 4:(g + 1) * 4, 0:512])
                nc.sync.dma_start(wq_sb[:, g * 4:(g + 1) * 4, :],
                                  wq_view[:, g * 4:(g + 1) * 4, :])
            for w_sb, wd in ((wk_sb, wkT_d), (wv_sb, wvT_d)):
                wv_view = wd.ap().rearrange("(a p) m -> p a m", p=P)
                for g in range(4):
                    nc.sync.dma_start(w_sb[:, g * 4:(g + 1) * 4, :],
                                      wv_view[:, g * 4:(g + 1) * 4, :])
            nc.sync.dma_start(ident[:], ident_d.ap())
            for o in range(4):
                nc.sync.dma_start(mask01[:, o, :], mask01_d.ap()[o])

            for tt in range(TT):
                tsl = slice(tt * 512, (tt + 1) * 512)
                if tt == 0:
                    xt = xt0
                else:
                    xt = xp.tile([P, KC, 512], MMDT, name=f"xt_{tt}", tag="xt")
                    for g in range(4):
                        nc.sync.dma_start(xt[:, g * 4:(g + 1) * 4, :],
                                          xT_view[:, g * 4:(g + 1) * 4, tsl])
                cs_c = csp.tile([P, 512], MMDT, tag="csc", name=f"csc_{tt}")
                nc.sync.dma_start(cs_c[:], cosA_d.ap()[:, tsl])
                cs_s = csp.tile([P, 512], MMDT, tag="css", name=f"css_{tt}")
                nc.sync.dma_start(cs_s[:], sinA_d.ap()[:, tsl])

                for w_sb, dst in ((wq_sb, qT_all), (wk_sb, kT_all)):
                    for mt in range(HPC):
                        ps = pp.tile([P, 512], dt.float32, tag="qk", bufs=6,
                                     name=f"psA_{tt}_{mt}")
                        for kc in range(KC):
                            nc.tensor.matmul(
                                ps[:], w_sb[:, kc, mt * P:(mt + 1) * P],
                                xt[:, kc, :],
                                start=(kc == 0), stop=(kc == KC - 1))
                        # rope: y = raw*cosA + halfswap(raw)*sinA
                        t1 = tp.tile([P, 512], dt.float32, tag="t1",
                                     name=f"t1_{tt}_{mt}")
                        nc.vector.tensor_mul(t1[:], ps[:], cs_c[:])
                        t2 = tp.tile([P, 512], dt.float32, tag="t2",
                                     name=f"t2_{tt}_{mt}")
                        nc.vector.tensor_mul(t2[0:64, :], ps[64:128, :],
                                             cs_s[0:64, :])
                        nc.vector.tensor_mul(t2[64:128, :], ps[0:64, :],
                                             cs_s[64:128, :])
                        nc.vector.tensor_add(dst[:, mt, tsl], t1[:], t2[:])

                # v computed transposed (N=512 matmuls), then PE-transposed
                # back to natural [token, d] layout for the PV stationary
                for mt in range(HPC):
                    ps = pp.tile([P, 512], dt.float32, tag="qk", bufs=6,
                                 name=f"psVT_{tt}_{mt}")
                    for kc in range(KC):
                        nc.tensor.matmul(
                            ps[:], wv_sb[:, kc, mt * P:(mt + 1) * P],
                            xt[:, kc, :],
                            start=(kc == 0), stop=(kc == KC - 1))
                    vT_sb = vtp.tile([P, 512], MMDT, tag="vts",
                                     name=f"vts_{tt}_{mt}")
                    nc.scalar.copy(vT_sb[:], ps[:])
                    for js in range(4):
                        pst = pp.tile([P, P], MMDT, tag="tp", bufs=2,
                                      name=f"pst_{tt}_{mt}_{js}")
                        nc.tensor.transpose(pst[:], vT_sb[:, js * P:(js + 1) * P],
                                            ident[:])
                        dst_v = v_all[:, tt * 4 + js, mt * P:(mt + 1) * P]
                        if js % 2 == 0:
                            nc.scalar.copy(dst_v, pst[:])
                        else:
                            nc.vector.tensor_copy(dst_v, pst[:])

        # ---------------- Phase B + C: attention, A2A, out-projection ------
        with tc.tile_pool(name="pb_p", bufs=6) as ppool, \
             tc.tile_pool(name="pb_nm", bufs=2) as nmp, \
             tc.tile_pool(name="pc_w", bufs=4) as pwp, \
             tc.tile_pool(name="pc_o", bufs=3) as ocp, \
             tc.tile_pool(name="pb_ps", bufs=1, space="PSUM") as pb:

            def attention_pair(b, hl):
                kT_h = kT_all[:, hl, b * T:(b + 1) * T]
                qT_h = qT_all[:, hl, b * T:(b + 1) * T]
                v_h = v_all[:, b * 16:(b + 1) * 16, hl * D:(hl + 1) * D]

                for qt in range(QT):
                    qTt = qT_h[:, qt * 512:(qt + 1) * 512]
                    n = 4 * (qt + 1)
                    smps = pb.tile([P, 512], dt.float32, tag="sm", bufs=2,
                                   name=f"sm_{b}_{hl}_{qt}")
                    pvps = pb.tile([P, 512], dt.float32, tag="pv", bufs=2,
                                   name=f"pv_{b}_{hl}_{qt}")

                    sc_tiles = {}

                    def emit_sc2(jp, _qt=qt, _q=qTt, _n=n, _sc=sc_tiles,
                                 _b=b, _hl=hl):
                        # two key chunks per 2-bank PSUM tile -> one exp
                        ps = pb.tile([P, 2, 512], dt.float32, tag="sc", bufs=2,
                                     name=f"sc_{_b}_{_hl}_{_qt}_{jp}")
                        for u in (0, 1):
                            jc = 2 * jp + u
                            nc.tensor.matmul(ps[:, u, :],
                                             kT_h[:, jc * P:(jc + 1) * P],
                                             _q[:], start=True, stop=True)
                        _sc[jp] = ps

                    pT_tiles = {}

                    def emit_exp(jp, _qt=qt, _n=n, _b=b, _hl=hl):
                        scps = sc_tiles.pop(jp)
                        pT2 = ppool.tile([P, 2, 512], MMDT, tag="pT",
                                         name=f"pT_{_b}_{_hl}_{_qt}_{jp}")
                        nc.scalar.activation(pT2[:], scps[:],
                                             mybir.ActivationFunctionType.Exp,
                                             scale=SCALE)
                        # diagonal-band chunks: zero the causally-masked
                        # entries with a 0/1 mask on the DVE (cheaper than
                        # masking matmuls on the PE)
                        for u in (0, 1):
                            jc = 2 * jp + u
                            if jc >= _n - 4:
                                o = jc - (_n - 4)
                                nc.vector.tensor_mul(pT2[:, u, :], pT2[:, u, :],
                                                     mask01[:, o, :])
                        pT_tiles[jp] = pT2

                    # two chunk-pairs of lookahead so exp latency stays off
                    # the PE critical path
                    emit_sc2(0)
                    emit_exp(0)
                    if n // 2 > 1:
                        emit_sc2(1)
                        emit_exp(1)
                    for jp in range(n // 2):
                        pT2 = pT_tiles.pop(jp)
                        for u in (0, 1):
                            jc = 2 * jp + u
                            nc.tensor.matmul(smps[:], ones_r[:], pT2[:, u, :],
                                             start=(jc == 0), stop=(jc == n - 1))
                            nc.tensor.matmul(pvps[:], v_h[:, jc, :],
                                             pT2[:, u, :],
                                             start=(jc == 0), stop=(jc == n - 1))
                        if jp + 2 < n // 2:
                            emit_sc2(jp + 2)
                            emit_exp(jp + 2)

                    # normalize: all 128 partitions of smps hold the colsum,
                    # so reciprocal of the full tile is a pre-broadcast scale
                    rec = nmp.tile([P, 512], dt.float32, tag="rec",
                                   name=f"rec_{b}_{hl}_{qt}")
                    nc.vector.reciprocal(rec[:], smps[:])
                    yt = nmp.tile([P, 512], MMDT, tag="yt",
                                  name=f"yt_{b}_{hl}_{qt}")
                    nc.vector.tensor_mul(yt[:], pvps[:], rec[:])
                    nc.sync.dma_start(chs[b][hl][2 * qt][:, :], yt[:, 0:256])
                    nc.sync.dma_start(chs[b][hl][2 * qt + 1][:, :],
                                      yt[:, 256:512])

                nc.gpsimd.collective_compute(
                    "AllToAll", mybir.AluOpType.bypass,
                    replica_groups=[list(range(NCORES))],
                    ins=[chs[b][hl].opt()], outs=[yos[b][hl].opt()],
                )

            def emit_gathers(hlh, b, split=False):
                # A2A-gated: emit only where nothing latency-critical queues
                # behind them on the same DMA queue. split=True gathers rank
                # by rank so the consumer's r=0 matmul starts ~2.5us sooner.
                yv = yos[b][hlh][:].rearrange("(r p) t -> p r t", p=P)
                if split:
                    for r in range(NCORES):
                        nc.sync.dma_start(
                            yAB[:, hlh, r, b * 256:(b + 1) * 256], yv[:, r, :])
                else:
                    nc.sync.dma_start(yAB[:, hlh, :, b * 256:(b + 1) * 256],
                                      yv[:])

            pw_sb = {}

            def load_pw(hlh):
                # ungated streaming loads; fully resident (no ring waits)
                for co in range(KC):
                    pw = pwp.tile([P, NCORES, P], MMDT, tag=f"pw{hlh}",
                                  bufs=KC, name=f"pw_{hlh}_{co}")
                    nc.sync.dma_start(pw[:], pwB_d.ap()[co, hlh])
                    pw_sb[(hlh, co)] = pw

            def cpass0():
                """hl=0 half of the output projection -> bf16 partials."""
                for cp in range(KC // 2):
                    pso = pb.tile([P, 2, 512], dt.float32, tag="sc", bufs=2,
                                  name=f"pso_0_{cp}")
                    for u in (0, 1):
                        co = 2 * cp + u
                        pw = pw_sb[(0, co)]
                        for r in range(NCORES):
                            nc.tensor.matmul(pso[:, u, :], pw[:, r, :],
                                             yAB[:, 0, r, :],
                                             start=(r == 0),
                                             stop=(r == NCORES - 1))
                    for u in (0, 1):
                        co = 2 * cp + u
                        if u == 0:
                            nc.scalar.copy(cpart[:, co, :], pso[:, u, :])
                        else:
                            nc.vector.tensor_copy(cpart[:, co, :], pso[:, u, :])

            def cpass1():
                """hl=1 half of the output projection, combined with the
                hl=0 partials and written out."""
                for cp in range(KC // 2):
                    pso = pb.tile([P, 2, 512], dt.float32, tag="sc", bufs=2,
                                  name=f"pso1_{cp}")
                    for u in (0, 1):
                        co = 2 * cp + u
                        pw = pw_sb[(1, co)]
                        for r in range(NCORES):
                            nc.tensor.matmul(pso[:, u, :], pw[:, r, :],
                                             yAB[:, 1, r, :],
                                             start=(r == 0),
                                             stop=(r == NCORES - 1))
                    for u in (0, 1):
                        co = 2 * cp + u
                        oo = ocp.tile([P, 512], MMDT, tag="oo", bufs=3,
                                      name=f"oo_{co}")
                        nc.vector.tensor_add(oo[:], pso[:, u, :],
                                             cpart[:, co, :])
                        nc.sync.dma_start(
                            outT_d.ap()[co * P:(co + 1) * P, :], oo[:])

            attention_pair(0, 0)
            attention_pair(1, 0)
            attention_pair(0, 1)
            load_pw(0)
            load_pw(1)
            attention_pair(1, 1)
            # all gathers sit here: nothing latency-critical queues behind
            # them, and the first-collective consumer is as late as possible
            # (max start-skew absorption)
            emit_gathers(0, 0)
            emit_gathers(0, 1)
            emit_gathers(1, 0)
            emit_gathers(1, 1)
            cpass0()                   # covers the a2a(1,1) latency
            cpass1()

    nc.compile()
    return nc


def _host_reference(x, weights, cos, sin, mask, use_lora):
    """Numpy fallback for inputs outside the optimized assumptions."""
    (q_w, q_b, q_A, q_B, k_w, k_b, k_A, k_B,
     v_w, v_b, v_A, v_B, p_w, p_b, p_A, p_B) = weights

    def lin(xx, w, b, A, Bm):
        out = xx @ w.T + b
        if use_lora:
            out = out + (xx @ A) @ Bm
        return out

    def rope(t):
        x1, x2 = t[..., ::2], t[..., 1::2]
        y = np.stack((x1 * cos - x2 * sin, x1 * sin + x2 * cos), axis=-1)
        return y.reshape(t.shape)

    Bs, Tl, Cd = x.shape
    q = lin(x, q_w, q_b, q_A, q_B).reshape(Bs, Tl, H, D).transpose(0, 2, 1, 3)
    k = lin(x, k_w, k_b, k_A, k_B).reshape(Bs, Tl, H, D).transpose(0, 2, 1, 3)
    v = lin(x, v_w, v_b, v_A, v_B).reshape(Bs, Tl, H, D).transpose(0, 2, 1, 3)
    q, k = rope(q), rope(k)
    s = np.einsum('bhqd,bhkd->bhqk', q, k) / np.sqrt(D)
    s = np.where(mask, s, -np.inf)
    s = s - s.max(axis=-1, keepdims=True)
    p = np.exp(s)
    p /= p.sum(axis=-1, keepdims=True)
    o = np.einsum('bhqk,bhkd->bhqd', p, v).transpose(0, 2, 1, 3).reshape(Bs, Tl, Cd)
    return lin(o, p_w, p_b, p_A, p_B).astype(np.float32)


def kernel(**inputs):
    x = np.asarray(inputs["x"], np.float32)
    cos = np.asarray(inputs["cos"], np.float32)
    sin = np.asarray(inputs["sin"], np.float32)
    mask = np.asarray(inputs["mask"])
    use_lora = int(np.asarray(inputs["use_lora"]))
    ws = {}
    for nm in ("q", "k", "v", "p"):
        for suf in ("w", "b", "A", "B"):
            ws[f"{nm}_{suf}"] = np.asarray(inputs[f"{nm}_{suf}"], np.float32)

    causal = bool((mask == np.tril(np.ones((T, T), bool))).all())
    zero_bias = all(not ws[f"{nm}_b"].any() for nm in ("q", "k", "v", "p"))
    if not (causal and zero_bias and x.shape == (B, T, C)):
        weights = tuple(ws[f"{nm}_{suf}"] for nm in ("q", "k", "v", "p")
                        for suf in ("w", "b", "A", "B"))
        return _host_reference(x, weights, cos, sin, mask, use_lora)

    # effective (LoRA-folded) transposed weights: out = x @ W_eff.T,
    # W_eff.T = w.T + A @ B
    effT = {}
    for nm in ("q", "k", "v", "p"):
        wt = ws[f"{nm}_w"].T.copy()
        if use_lora:
            wt += ws[f"{nm}_A"] @ ws[f"{nm}_B"]
        effT[nm] = np.ascontiguousarray(wt, np.float32)

    xT = np.ascontiguousarray(x.reshape(BT, C).T)

    # sigma: within each head reorder out-features to [evens, odds] so the
    # rope pair-rotation becomes a partition half-swap
    perm = np.concatenate([np.arange(0, D, 2), np.arange(1, D, 2)])
    cosT = cos.T.astype(np.float32)          # [64, T]
    sinT = sin.T.astype(np.float32)
    cosA = np.tile(np.vstack([cosT, cosT]), (1, B))          # [128, B*T]
    sinA = np.tile(np.vstack([-sinT, sinT]), (1, B))

    # causal 0/1 mask for the 4 diagonal-band key chunks of each query tile:
    # mask01[o, k, q] = 1 iff key k + 128*o <= query q (multiplies exp'd
    # scores on the DVE)
    kk = np.arange(P)[:, None]
    qq = np.arange(512)[None, :]
    mask01 = np.stack([(kk + 128 * o <= qq).astype(np.float32)
                       for o in range(4)])

    # output projection weight, blocked [co, hl, p, r, m]: y-feature block for
    # head (r, hl) contracts against rows (2r+hl)*128+p of effT["p"]
    pwB = np.ascontiguousarray(
        effT["p"].reshape(NCORES, HPC, P, KC, P)     # [r, hl, p, co, m]
        .transpose(3, 1, 2, 0, 4))                   # [co, hl, p, r, m]

    ident = np.eye(P, dtype=np.float32)

    global _PROGRAM
    if _PROGRAM is None:
        _PROGRAM = _build_program()
    nc = _PROGRAM

    mmnp = mybir.dt.np(MMDT)

    in_maps = []
    xT_mm = xT.astype(mmnp)
    pwB_mm = pwB.astype(mmnp)
    for c in range(NCORES):
        cols = slice(c * HPC * D, (c + 1) * HPC * D)
        wqT = effT["q"][:, cols].copy()
        wkT = effT["k"][:, cols].copy()
        for hl in range(HPC):
            sl = slice(hl * D, (hl + 1) * D)
            wqT[:, sl] = wqT[:, sl][:, perm]
            wkT[:, sl] = wkT[:, sl][:, perm]
        in_maps.append({
            "xT": xT_mm,
            "wqT": np.ascontiguousarray(wqT).astype(mmnp),
            "wkT": np.ascontiguousarray(wkT).astype(mmnp),
            "wvT": np.ascontiguousarray(effT["v"][:, cols]).astype(mmnp),
            "pwB": pwB_mm,
            "cosA": cosA.astype(mmnp),
            "sinA": sinA.astype(mmnp),
            "mask01": mask01.astype(mmnp),
            "ident": ident.astype(mmnp),
        })

    res = run_bass_kernel_spmd(nc, in_maps, list(range(NCORES)))

    out = np.empty((BT, C), np.float32)
    for c in range(NCORES):
        oT = np.asarray(res.results[c]["outT"], np.float32)   # [2048, 512]
        out[c * 256:(c + 1) * 256, :] = oT[:, 0:256].T             # b = 0
        out[T + c * 256:T + (c + 1) * 256, :] = oT[:, 256:512].T   # b = 1
    return out.reshape(B, T, C)
